# revision 13
# baseline (speedup 1.0000x reference)
"""Trainium2 Bass kernel for degree-3 B-spline basis evaluation (DF=16).

Math: with x = t * 13/(1+1e-7), output column i equals the cardinal cubic
B-spline N3(x + 3 - i), where N3 (support [0,4]) is evaluated via the
symmetric two-cube form

    a  = |z - 2|
    N3 = (2-a)+^3 / 6  -  (2/3) (1-a)+^3

Boundary columns {0,1,2,13,14,15} get corrections that are linear
combinations of the one-sided truncated cubes (1-x)+^3, (2-x)+^3 (left) and
(x-12)+^3, (x-11)+^3 (right); coefficients fitted in f64 against the
reference Cox-de Boor recursion (max model error vs f32 reference ~2.7e-6).

Sharding: ts is flattened to [4194304] and split evenly across 8 cores
(embarrassingly data-parallel, no communication).
"""
import numpy as np

import concourse.bacc as bacc
import concourse.bass as bass
import concourse.mybir as mybir
import concourse.tile as tile
from concourse.bass_utils import run_bass_kernel_spmd

F32 = mybir.dt.float32
ALU = mybir.AluOpType
ACTF = mybir.ActivationFunctionType

B, S, DF = 32, 131072, 16
N_CORES = 8
TOTAL = B * S                      # 4194304
N_EL = TOTAL // N_CORES            # 524288 elements per core
P = 128
FREE = N_EL // P                   # 4096 elements per partition
L = 256                            # elements per partition per tile
NT = FREE // L                     # 16 tiles
SCALE = 13.0 / (1.0 + 1e-7)

# Boundary correction coefficients on (1-x)+^3, (2-x)+^3 (left) and
# (x-12)+^3, (x-11)+^3 (right); fitted vs the reference (see study).
CO_0 = (0.8333333504420133,)
CO_1 = (-1.3333334132712784, 0.08333334011939457)
CO_2 = (0.500000040934657, -0.08333332976517654)
CO_15 = (0.8333358648246846,)
CO_14 = (-1.3333370986744204, 0.08333348733671964)
CO_13 = (0.5000018172429225, -0.08333357268446573)

_CACHE: dict = {}


def _build_program() -> bass.Bass:
    nc = bacc.Bacc()

    ts_in = nc.declare_dram_parameter("ts_in", [P, FREE], F32, isOutput=False)
    c16_in = nc.declare_dram_parameter("c16", [P, L * DF], F32, isOutput=False)
    out = nc.declare_dram_parameter("out", [P, FREE * DF], F32, isOutput=True)

    with tile.TileContext(nc) as tc:
        import contextlib
        with contextlib.ExitStack() as ctx:
            const_pool = ctx.enter_context(tc.tile_pool(name="const", bufs=1))
            big = ctx.enter_context(tc.tile_pool(name="big", bufs=6))
            outp = ctx.enter_context(tc.tile_pool(name="outp", bufs=2))
            nar = ctx.enter_context(tc.tile_pool(name="nar", bufs=2))

            # Resident input + constants
            ts_sb = const_pool.tile([P, FREE], F32, tag="ts_sb")
            nc.sync.dma_start(ts_sb[:], ts_in[:])
            c16 = const_pool.tile([P, L * DF], F32, tag="c16")
            nc.sync.dma_start(c16[:], c16_in[:])
            # bias const for ACT Square (only 0.0/1.0 are pre-registered)
            b2 = const_pool.tile([P, 1], F32, tag="b2")
            nc.vector.memset(b2[:], 2.0)

            for k in range(NT):
                W = L * DF
                # x1 = SCALE*t + 1   (narrow [P, L])
                x1 = nar.tile([P, L], F32, tag="x1")
                nc.vector.tensor_scalar(
                    x1[:], ts_sb[:, k * L:(k + 1) * L], SCALE, 1.0,
                    ALU.mult, ALU.add)

                # w = x1 - i  (full [P, L*16]); c16 is the pre-tiled i-pattern
                w = big.tile([P, W], F32, tag="big")
                x1_b = x1[:].unsqueeze(2).broadcast_to((P, L, DF))
                w3 = w[:].rearrange("p (l c) -> p l c", c=DF)
                c163 = c16[:].rearrange("p (l c) -> p l c", c=DF)
                nc.vector.tensor_tensor(w3, x1_b, c163, ALU.subtract)

                # a = |w|
                a = big.tile([P, W], F32, tag="big")
                nc.scalar.activation(a[:], w[:], ACTF.Abs)

                # P1 = (2-a)^2 ; Q1 = min(a-2, 0) = -(2-a)+
                p1 = big.tile([P, W], F32, tag="big")
                nc.scalar.activation(p1[:], a[:], ACTF.Square, scale=-1.0,
                                     bias=b2[:])
                q1 = big.tile([P, W], F32, tag="big")
                nc.gpsimd.tensor_scalar(q1[:], a[:], 2.0, 0.0,
                                        ALU.subtract, ALU.min)
                # T1 = (P1 * -1/6) * Q1 = (2-a)+^3 / 6
                t1 = big.tile([P, W], F32, tag="big")
                nc.vector.scalar_tensor_tensor(
                    t1[:], p1[:], -1.0 / 6.0, q1[:], ALU.mult, ALU.mult)

                # P2 = (1-a)^2 ; Q2 = min(a-1, 0) = -(1-a)+
                p2 = big.tile([P, W], F32, tag="big")
                nc.scalar.activation(p2[:], a[:], ACTF.Square, scale=-1.0,
                                     bias=1.0)
                q2 = big.tile([P, W], F32, tag="big")
                nc.gpsimd.tensor_scalar(q2[:], a[:], 1.0, 0.0,
                                        ALU.subtract, ALU.min)
                # T2 = (P2 * 2/3) * Q2 = -(2/3)(1-a)+^3
                t2 = big.tile([P, W], F32, tag="big")
                nc.vector.scalar_tensor_tensor(
                    t2[:], p2[:], 2.0 / 3.0, q2[:], ALU.mult, ALU.mult)

                o = outp.tile([P, W], F32, tag="outp")
                nc.vector.tensor_tensor(o[:], t1[:], t2[:], ALU.add)

                # ---- boundary corrections ----
                # left cubes: d = min(x1-(k+1),0) = -(k-x)+ ; c = d^3 = -(k-x)+^3
                d1 = nar.tile([P, L], F32, tag="d1")
                nc.gpsimd.tensor_scalar(d1[:], x1[:], 2.0, 0.0,
                                        ALU.subtract, ALU.min)
                s1 = nar.tile([P, L], F32, tag="s1")
                nc.scalar.activation(s1[:], d1[:], ACTF.Square)
                c1 = nar.tile([P, L], F32, tag="c1")
                nc.gpsimd.tensor_tensor(c1[:], s1[:], d1[:], ALU.mult)

                d2 = nar.tile([P, L], F32, tag="d2")
                nc.gpsimd.tensor_scalar(d2[:], x1[:], 3.0, 0.0,
                                        ALU.subtract, ALU.min)
                s2 = nar.tile([P, L], F32, tag="s2")
                nc.scalar.activation(s2[:], d2[:], ACTF.Square)
                c2 = nar.tile([P, L], F32, tag="c2")
                nc.gpsimd.tensor_tensor(c2[:], s2[:], d2[:], ALU.mult)

                # right cubes: e = max(x1-(14-k),0) = (x-(13-k))+ ; ce = e^3
                e1 = nar.tile([P, L], F32, tag="e1")
                nc.gpsimd.tensor_scalar(e1[:], x1[:], 13.0, 0.0,
                                        ALU.subtract, ALU.max)
                se1 = nar.tile([P, L], F32, tag="se1")
                nc.scalar.activation(se1[:], e1[:], ACTF.Square)
                ce1 = nar.tile([P, L], F32, tag="ce1")
                nc.gpsimd.tensor_tensor(ce1[:], se1[:], e1[:], ALU.mult)

                e2 = nar.tile([P, L], F32, tag="e2")
                nc.gpsimd.tensor_scalar(e2[:], x1[:], 12.0, 0.0,
                                        ALU.subtract, ALU.max)
                se2 = nar.tile([P, L], F32, tag="se2")
                nc.scalar.activation(se2[:], e2[:], ACTF.Square)
                ce2 = nar.tile([P, L], F32, tag="ce2")
                nc.gpsimd.tensor_tensor(ce2[:], se2[:], e2[:], ALU.mult)

                o3 = o[:].rearrange("p (l c) -> p l c", c=DF)

                # col 0: += CO_0[0]*(1-x)+^3 = -CO_0[0]*c1
                nc.vector.scalar_tensor_tensor(
                    o3[:, :, 0], c1[:], -CO_0[0], o3[:, :, 0],
                    ALU.mult, ALU.add)
                # col 1: += CO_1[0]*(1-x)+^3 + CO_1[1]*(2-x)+^3
                #       = -(c1*(A/B) + c2)*B
                m1 = nar.tile([P, L], F32, tag="m1")
                nc.vector.scalar_tensor_tensor(
                    m1[:], c1[:], CO_1[0] / CO_1[1], c2[:], ALU.mult, ALU.add)
                nc.vector.scalar_tensor_tensor(
                    o3[:, :, 1], m1[:], -CO_1[1], o3[:, :, 1],
                    ALU.mult, ALU.add)
                # col 2
                m2 = nar.tile([P, L], F32, tag="m2")
                nc.vector.scalar_tensor_tensor(
                    m2[:], c1[:], CO_2[0] / CO_2[1], c2[:], ALU.mult, ALU.add)
                nc.vector.scalar_tensor_tensor(
                    o3[:, :, 2], m2[:], -CO_2[1], o3[:, :, 2],
                    ALU.mult, ALU.add)
                # col 15: += CO_15[0]*ce1
                nc.vector.scalar_tensor_tensor(
                    o3[:, :, 15], ce1[:], CO_15[0], o3[:, :, 15],
                    ALU.mult, ALU.add)
                # col 14: += CO_14[0]*ce1 + CO_14[1]*ce2 = (ce1*(A/B)+ce2)*B
                m14 = nar.tile([P, L], F32, tag="m14")
                nc.vector.scalar_tensor_tensor(
                    m14[:], ce1[:], CO_14[0] / CO_14[1], ce2[:],
                    ALU.mult, ALU.add)
                nc.vector.scalar_tensor_tensor(
                    o3[:, :, 14], m14[:], CO_14[1], o3[:, :, 14],
                    ALU.mult, ALU.add)
                # col 13
                m13 = nar.tile([P, L], F32, tag="m13")
                nc.vector.scalar_tensor_tensor(
                    m13[:], ce1[:], CO_13[0] / CO_13[1], ce2[:],
                    ALU.mult, ALU.add)
                nc.vector.scalar_tensor_tensor(
                    o3[:, :, 13], m13[:], CO_13[1], o3[:, :, 13],
                    ALU.mult, ALU.add)

                nc.sync.dma_start(out[:, k * W:(k + 1) * W], o[:])

    nc.finalize()
    return nc


def _get_program() -> bass.Bass:
    if "nc" not in _CACHE:
        _CACHE["nc"] = _build_program()
    return _CACHE["nc"]


def kernel(ts: np.ndarray) -> np.ndarray:
    assert ts.shape == (B, S) and ts.dtype == np.float32
    nc = _get_program()

    flat = np.ascontiguousarray(ts).reshape(TOTAL)
    c16 = np.ascontiguousarray(
        np.broadcast_to(np.arange(DF, dtype=np.float32),
                        (P, L, DF)).reshape(P, L * DF))
    in_maps = []
    for c in range(N_CORES):
        shard = flat[c * N_EL:(c + 1) * N_EL].reshape(P, FREE)
        in_maps.append({"ts_in": np.ascontiguousarray(shard), "c16": c16})

    res = run_bass_kernel_spmd(nc, in_maps, core_ids=list(range(N_CORES)))
    parts = [r["out"].reshape(N_EL, DF) for r in res.results]
    return np.concatenate(parts, axis=0).reshape(B, S, DF)


# revision 15
# speedup vs baseline: 9.3252x; 9.3252x over previous
"""Trainium2 Bass kernel for degree-3 B-spline basis evaluation (DF=16).

Math: with x = t * 13/(1+1e-7) and a_i = |x + 1 - i| for output column i,
every interior column equals the cardinal cubic B-spline

    B_i = (2-a)+^3 / 6  -  (2/3)(1-a)+^3 .

Two custom fused DVE ops evaluate this in exactly two full-width passes:

  pass A:  q1s = (min(|x1 - i|, 2) - 2) * (-6^(-1/3))        (x1 = x+1)
  pass B:  out = q1s^3 + min(-4^(1/3)*q1s + (2/3)^(1/3), 0)^3

(The second leg reconstructs the width-1 clamp from q1s in-slice, so no
second |.| pass is needed.)

Boundary columns {0,1,2,13,14,15} are exact one-sided truncated-cube
combinations (coefficients ~{1, -2, 1/4, 3/2, -3/4, 1/6}, fitted in f64
against the reference Cox-de Boor recursion including its 1e-7 knot quirk);
they are rebuilt from narrow relu-cube passes and overwritten into the
output tile. Max model error vs the f32 reference: ~2.9e-6 (scale-rel).

Sharding: ts flattened to [4194304], split evenly across 8 cores
(embarrassingly data-parallel, no communication).
"""
import numpy as np

import concourse.bacc as bacc
import concourse.bass as bass
import concourse.dve_ops as dve_ops
import concourse.mybir as mybir
import concourse.tile as tile
from concourse.bass_utils import run_bass_kernel_spmd
from concourse.dve_ops import DveOp
from concourse.dve_spec import (
    AluOp as DAlu,
    Bin,
    C0,
    C1,
    Spec,
    Src0,
    Src1,
    Zero,
    _has_src1,
    lower,
    minn,
    relu,
    sq,
)
from concourse.dve_uop import DveOpSpec

F32 = mybir.dt.float32
ALU = mybir.AluOpType

B, S, DF = 32, 131072, 16
N_CORES = 8
TOTAL = B * S                      # 4194304
N_EL = TOTAL // N_CORES            # 524288 elements per core
P = 128
FREE = N_EL // P                   # 4096 elements per partition
L = 256                            # elements per partition per tile
NT = FREE // L                     # tiles per core
W = L * DF
SCALE = 13.0 / (1.0 + 1e-7)
K6 = 6.0 ** (-1.0 / 3.0)           # q1s scale
C0B = -(4.0 ** (1.0 / 3.0))        # pass-B leg-2 scale
C1B = (2.0 / 3.0) ** (1.0 / 3.0)   # pass-B leg-2 offset

# One-sided truncated-cube coefficients for boundary columns (see study2):
# col c = sum_k FITS[c][k] * cube_k, cubes (1-x)+^3,(2-x)+^3,(3-x)+^3 (left)
# or (x-12)+^3,(x-11)+^3,(x-10)+^3 (right).
FITS = {
    0: [1.0000000170155405],
    1: [-2.0000000799500466, 0.25000000679821277],
    2: [1.5000000407855558, -0.7499999964076991, 0.16666666444691997],
    13: [1.5000018175486398, -0.7500002393453886, 0.16666667442437125],
    14: [-2.0000037658853564, 0.2500001539605719],
    15: [1.0000025326505537],
}

_CACHE: dict = {}


def _register_op(name: str, spec: Spec, subdim: bool = False) -> DveOp:
    """Append a custom DVE op to dve_ops.OPS at runtime, with pinned shas
    computed the same way DveOp.compile does."""
    if name in dve_ops._SUB_OPCODE_FOR_NAME:
        for op in dve_ops.OPS:
            if op.name == name:
                return op
        raise RuntimeError(f"{name} mapped but not in OPS")
    row = dve_ops._CUSTOM_DVE_ROW_BASE + len(dve_ops.OPS)
    assert row < 0x20, "custom-DVE row field overflow"
    dve_ops._SUB_OPCODE_FOR_NAME[name] = row
    shas = {}
    for ver in ("v3", "v4"):
        tmp = DveOpSpec(name=name, opcode=row, uops=lower(spec, ver=ver),
                        rd1_en=_has_src1(spec))
        shas[ver] = tmp.sha(ver)
    op = DveOp(name, spec, subdim=subdim, uops_sha=shas)
    dve_ops.OPS.append(op)
    return op


_ad = Bin(DAlu.ABSOLUTE_DIFF, Src0, Src1)
OP_Q1S = _register_op(
    "ANT_BSPL_Q1S",
    Spec(body=(minn(_ad, C0) - C0) * C1,
         reference=lambda in0, in1, s0, s1, imm2:
             (np.minimum(np.abs(in0 - in1), s0) - s0) * s1))

_w = minn(Src0 * C0 + C1, Zero)
OP_COMB = _register_op(
    "ANT_BSPL_COMB",
    Spec(body=sq(Src0) * Src0 + sq(_w) * _w,
         reference=lambda in0, in1, s0, s1, imm2:
             in0 ** 3 + np.minimum(in0 * s0 + s1, 0) ** 3))

_r = relu(Src0 * C0 + C1)
OP_RCUBE = _register_op(
    "ANT_BSPL_RCUBE",
    Spec(body=sq(_r) * _r,
         reference=lambda in0, in1, s0, s1, imm2:
             np.maximum(in0 * s0 + s1, 0) ** 3))


def _build_program() -> bass.Bass:
    nc = bacc.Bacc()

    ts_in = nc.declare_dram_parameter("ts_in", [P, FREE], F32, isOutput=False)
    c16_in = nc.declare_dram_parameter("c16", [P, W], F32, isOutput=False)
    out = nc.declare_dram_parameter("out", [P, FREE * DF], F32, isOutput=True)

    with tile.TileContext(nc) as tc:
        import contextlib
        with contextlib.ExitStack() as ctx:
            const_pool = ctx.enter_context(tc.tile_pool(name="const", bufs=1))
            big = ctx.enter_context(tc.tile_pool(name="big", bufs=3))
            outp = ctx.enter_context(tc.tile_pool(name="outp", bufs=3))
            nar = ctx.enter_context(tc.tile_pool(name="nar", bufs=2))

            ts_sb = const_pool.tile([P, FREE], F32, tag="ts_sb")
            nc.sync.dma_start(ts_sb[:], ts_in[:])
            c16 = const_pool.tile([P, W], F32, tag="c16")
            nc.sync.dma_start(c16[:], c16_in[:])

            for k in range(NT):
                # x1 = SCALE*t + 1
                x1 = nar.tile([P, L], F32, tag="x1")
                nc.vector.tensor_scalar(
                    x1[:], ts_sb[:, k * L:(k + 1) * L], SCALE, 1.0,
                    ALU.mult, ALU.add)

                # pass A: q1s = (min(|x1 - i|, 2) - 2) * (-K6)
                q1s = big.tile([P, W], F32, tag="q1s")
                x1_b = x1[:].unsqueeze(2).broadcast_to((P, L, DF))
                c16_3 = c16[:].rearrange("p (l c) -> p l c", c=DF)
                q1s_3 = q1s[:].rearrange("p (l c) -> p l c", c=DF)
                nc.vector._custom_dve(OP_Q1S, out=q1s_3, in0=x1_b, in1=c16_3,
                                      s0=2.0, s1=-K6)

                # pass B: o = q1s^3 + min(C0B*q1s + C1B, 0)^3
                o = outp.tile([P, W], F32, tag="outp")
                nc.vector._custom_dve(OP_COMB, out=o[:], in0=q1s[:],
                                      s0=C0B, s1=C1B)

                # boundary cubes (narrow, contiguous)
                cb = {}
                for nm, c0_, c1_ in [("L1", -1.0, 2.0), ("L2", -1.0, 3.0),
                                     ("L3", -1.0, 4.0), ("R1", 1.0, -13.0),
                                     ("R2", 1.0, -12.0), ("R3", 1.0, -11.0)]:
                    cbt = nar.tile([P, L], F32, tag=nm)
                    nc.vector._custom_dve(OP_RCUBE, out=cbt[:], in0=x1[:],
                                          s0=c0_, s1=c1_)
                    cb[nm] = cbt

                # combine + write into interleaved [P, L, 3] staging tiles
                fl = nar.tile([P, L * 3], F32, tag="fl")
                fr = nar.tile([P, L * 3], F32, tag="fr")
                fl3 = fl[:].rearrange("p (l c) -> p l c", c=3)
                fr3 = fr[:].rearrange("p (l c) -> p l c", c=3)

                for f3, names, cols in ((fl3, ("L1", "L2", "L3"), (0, 1, 2)),
                                        (fr3, ("R1", "R2", "R3"),
                                         (13, 14, 15))):
                    for slot, col in enumerate(cols):
                        co = FITS[col]
                        m = cb[names[0]]
                        for idx in range(1, len(co)):
                            mn = nar.tile([P, L], F32, tag=f"m{col}_{idx}")
                            nc.vector.scalar_tensor_tensor(
                                mn[:], m[:], co[idx - 1] / co[idx],
                                cb[names[idx]][:], ALU.mult, ALU.add)
                            m = mn
                        nc.vector.tensor_scalar(
                            f3[:, :, slot], m[:], co[-1], None, ALU.mult)

                # overwrite boundary columns in the output tile
                o3 = o[:].rearrange("p (l c) -> p l c", c=DF)
                nc.vector.tensor_copy(o3[:, :, 0:3], fl3)
                nc.vector.tensor_copy(o3[:, :, 13:16], fr3)

                nc.sync.dma_start(out[:, k * W:(k + 1) * W], o[:])

    nc.finalize()
    return nc


def _get_program() -> bass.Bass:
    if "nc" not in _CACHE:
        _CACHE["nc"] = _build_program()
    return _CACHE["nc"]


def _make_c16() -> np.ndarray:
    return np.ascontiguousarray(
        np.broadcast_to(np.arange(DF, dtype=np.float32),
                        (P, L, DF)).reshape(P, W))


def kernel(ts: np.ndarray) -> np.ndarray:
    assert ts.shape == (B, S) and ts.dtype == np.float32
    nc = _get_program()

    flat = np.ascontiguousarray(ts).reshape(TOTAL)
    c16 = _make_c16()
    in_maps = []
    for c in range(N_CORES):
        shard = flat[c * N_EL:(c + 1) * N_EL].reshape(P, FREE)
        in_maps.append({"ts_in": np.ascontiguousarray(shard), "c16": c16})

    res = run_bass_kernel_spmd(nc, in_maps, core_ids=list(range(N_CORES)))
    parts = [r["out"].reshape(N_EL, DF) for r in res.results]
    return np.concatenate(parts, axis=0).reshape(B, S, DF)
